# revision 1
# baseline (speedup 1.0000x reference)
"""Trainium2 Bass kernel for nn_Downsampler: depthwise 4x4 conv, stride 4,
VALID padding, one shared (runtime) 4x4 kernel across all channels.

  x: (16, 8, 1024, 1024) f32, kernel: (4, 4) f32 -> out: (16, 8, 256, 256) f32

Sharding: pure data parallel over batch N=16 -> 2 batches per core on 8 cores.

Math: out[o, j] = sum_{di,dj} k[di,dj] * x[4o+di, 4j+dj], rows flattened over
(n, c, h) since every image row has W=1024 and slabs never straddle an (n, c)
boundary (1024 rows per image, slab = 512 rows).

Two-stage implementation, per slab of 512 input rows held as an SBUF tile
[128, 4096] (partition p, quarter d -> row 512*s + 128*d + p):

1. Horizontal pass (W-downsample). Row r uses kernel row k[r%4, :], and
   r%4 == p%4 in every quarter, so the weights are a per-partition scalar
   ks[p, dj] = kernel[p%4, dj]:
       hp[p, (d, j)] = sum_dj ks[p, dj] * xt[p, (d, 4j+dj)]
   4 fused multiply-accumulates spread across engines: ScalarE
   ACTIVATE(Copy, scale) for dj=0 and (even slabs) dj=1, GpSimd
   tensor_tensor for (odd slabs) dj=1 and the hp+hp2 combine, VectorE
   scalar_tensor_tensor for dj=2,3.

2. Vertical pass (H-downsample) on the TensorEngine with a 0/1 selection
   matrix sel[p, m] = (p//4 == m), contracting the 4 rows of each group:
       psum[m, (d, j)] = sum_p sel[p, m] * hp[p, (d, j)]
   Dense fp32 rhs, N=512 per matmul; output row = 32*d + m.

PSUM eviction (ScalarE) and the output DMA for slab s are emitted TAIL_LAG
slabs later: engine queues are in-order, so an eagerly-emitted evict(s)
would sit at the head of ScalarE's queue waiting on matmul(s) and stall
slab s+1's first multiply behind it, serializing the pipeline. The output
DMA rides the ScalarE HWDGE ring, keeping the SP ring a pure input stream.

All arithmetic is fp32; the result matches the f32 jax reference to
rounding (~1e-7 rel).
"""

import json
from contextlib import ExitStack

import numpy as np

import concourse.bass as bass
import concourse.mybir as mybir
from concourse.tile import TileContext
from concourse.bass_utils import run_bass_kernel_spmd

N, C, H, W = 16, 8, 1024, 1024
F = 4
N_CORES = 8
R = (N // N_CORES) * C * H  # input rows per core (16384)
WO = W // F  # output row length (256)


def _split_excess_waits(bir_bytes: bytes, max_waits: int = 1) -> bytes:
    """The public neuronxcc walrus supports at most ONE sync wait per
    instruction; hoist excess waits onto NoOps inserted just before."""
    m = json.loads(bir_bytes)

    def fix(blocks):
        for bb in blocks:
            out = []
            for ins in bb.get("instructions", []):
                si = ins.get("sync_info")
                waits = (si or {}).get("on_wait") or []
                if len(waits) > max_waits:
                    extra = waits[:-max_waits]
                    si["on_wait"] = waits[-max_waits:]
                    for i in range(0, len(extra), max_waits):
                        out.append(
                            {
                                "debug": ins.get("debug", 0),
                                "engine": ins["engine"],
                                "ins": [],
                                "outs": [],
                                "name": f"{ins['name']}-ws{i}",
                                "opcode": "NoOp",
                                "sync_info": {
                                    "on_update": [],
                                    "on_wait": extra[i : i + max_waits],
                                },
                            }
                        )
                out.append(ins)
            bb["instructions"] = out
            fix(bb.get("blocks", []))

    for f in m["functions"]:
        fix(f["blocks"])
    return json.dumps(m).encode()


def _make_ks(kernel: np.ndarray) -> np.ndarray:
    """Per-partition horizontal weights [128, 4]: ks[p, dj] = kernel[p%4, dj]."""
    kernel = np.asarray(kernel, dtype=np.float32)
    assert kernel.shape == (F, F)
    return np.ascontiguousarray(kernel[np.arange(128) % F, :])


def _make_sel() -> np.ndarray:
    """Vertical selection matmul weights [128, 32]: sel[p, m] = (p//4 == m)."""
    p = np.arange(128)
    return (p[:, None] // F == np.arange(32)[None, :]).astype(np.float32)


def _build_nc(
    rows: int, xt_bufs: int = 3, m_bufs: int = 5, psum_bufs: int = 3, o_bufs: int = 4
) -> bass.Bass:
    assert rows % 2048 == 0
    n_groups = rows // 2048  # 4 slabs of 512 rows per PSUM group

    nc = bass.Bass("TRN2", target_bir_lowering=False, debug=False)
    x = nc.dram_tensor("x", [rows, W], mybir.dt.float32, kind="ExternalInput")
    ks = nc.dram_tensor("ks", [128, F], mybir.dt.float32, kind="ExternalInput")
    sel = nc.dram_tensor("sel", [128, 32], mybir.dt.float32, kind="ExternalInput")
    y = nc.dram_tensor("y", [rows // F, WO], mybir.dt.float32, kind="ExternalOutput")

    mult = mybir.AluOpType.mult
    add = mybir.AluOpType.add

    with TileContext(nc) as tc:
        with ExitStack() as ctx:
            const_pool = ctx.enter_context(tc.tile_pool(name="const_pool", bufs=1))
            kst = const_pool.tile([128, F], mybir.dt.float32)
            nc.sync.dma_start(kst[:], ks.ap())
            selt = const_pool.tile([128, 32], mybir.dt.float32)
            nc.sync.dma_start(selt[:], sel.ap())

            # keep-warm scratch: a dummy matmul per slab fills the PE's
            # inter-slab wait gaps so the HAM clock gate stays at K=8/8
            # (2.4 GHz) instead of re-throttling to 1.2 GHz
            wp_pool = ctx.enter_context(
                tc.tile_pool(name="wp_pool", bufs=1, space="PSUM")
            )
            warm_pt = wp_pool.tile([32, 512], mybir.dt.float32)
            warm_src = const_pool.tile([128, 256], mybir.dt.float32)
            nc.vector.memset(warm_src[:], 1.0)

            x_pool = ctx.enter_context(tc.tile_pool(name="x_pool", bufs=xt_bufs))
            m_pool = ctx.enter_context(tc.tile_pool(name="m_pool", bufs=m_bufs))
            ps_pool = ctx.enter_context(
                tc.tile_pool(name="ps_pool", bufs=psum_bufs, space="PSUM")
            )
            o_pool = ctx.enter_context(tc.tile_pool(name="o_pool", bufs=o_bufs))

            TAIL_LAG = 1  # groups (4 slabs each)
            pending: list = []

            def emit_tail(g: int, pt) -> None:
                # evict 4 slabs' PSUM -> SBUF at once (DMA cannot read PSUM)
                ot = o_pool.tile([128, 4 * WO], mybir.dt.float32, name="ot")
                nc.scalar.copy(ot[:], pt[:])
                # ot[32q+m, (d, j)] -> y row (4g+q)*128 + 32*d + m, one DMA
                # per slab (the AP balancer caps at 3 dims). The output DMAs
                # ride the ScalarE HWDGE ring (SP ring stays a pure input
                # stream).
                for q in range(4):
                    base = (4 * g + q) * 128
                    dst = y.ap()[base : base + 128, :].rearrange(
                        "(d m) j -> m d j", d=4
                    )
                    nc.scalar.dma_start(
                        dst,
                        ot[32 * q : 32 * q + 32, :].rearrange(
                            "m (d j) -> m d j", d=4
                        ),
                    )

            for g in range(n_groups):
                # one PSUM tile holds 4 slabs via matmul col-tiling: slab
                # q's output lands on partitions 32q..32q+32
                pt = ps_pool.tile([128, 4 * WO], mybir.dt.float32, name="pt")
                for q in range(4):
                    s = 4 * g + q
                    if q % 2 == 0:
                        # one input DMA covers TWO slabs (fewer trigger
                        # gaps in the SP input stream)
                        xt2 = x_pool.tile(
                            [128, 8 * W], mybir.dt.float32, name="xt"
                        )
                        src = x.ap()[s * 512 : (s + 2) * 512, :].rearrange(
                            "(d p) w -> p d w", p=128
                        )
                        nc.sync.dma_start(
                            xt2[:].rearrange("p (d w) -> p d w", d=8), src
                        )
                    half = q % 2
                    # [128, d, j, dj]: element = xt[p, d*W + 4j + dj]
                    xv = xt2[:].rearrange("p (d j q) -> p d j q", d=8, q=F)[
                        :, 4 * half : 4 * half + 4, :, :
                    ]

                    m0 = m_pool.tile([128, 4 * WO], mybir.dt.float32, name="m0")
                    m1 = m_pool.tile([128, 4 * WO], mybir.dt.float32, name="m1")
                    m2 = m_pool.tile([128, 4 * WO], mybir.dt.float32, name="m2")
                    m0v = m0[:].rearrange("p (d j) -> p d j", d=4)
                    m1v = m1[:].rearrange("p (d j) -> p d j", d=4)
                    m2v = m2[:].rearrange("p (d j) -> p d j", d=4)

                    # m0 = ks0*x(0), m1 = ks1*x(1) on ScalarE (strided mults
                    # are cheap on ACT: ~1.24us/1024)
                    nc.scalar.activation(
                        m0v, xv[:, :, :, 0],
                        mybir.ActivationFunctionType.Copy, scale=kst[:, 0:1],
                    )
                    if s % 2 == 0:
                        nc.scalar.activation(
                            m1v, xv[:, :, :, 1],
                            mybir.ActivationFunctionType.Copy, scale=kst[:, 1:2],
                        )
                    else:
                        nc.gpsimd.tensor_tensor(
                            m1v,
                            xv[:, :, :, 1],
                            kst[:, 1:2].broadcast_to([128, 4, WO]),
                            mult,
                        )
                    # m0 += m1 on GpSimd
                    nc.gpsimd.tensor_tensor(m0[:], m0[:], m1[:], add)
                    # m2 = ks2*x(2) on VectorE tensor_scalar (2x mode, ~745ns)
                    nc.vector.tensor_scalar(
                        m2v, xv[:, :, :, 2], kst[:, 2:3], None, mult
                    )
                    # m2 += ks3*x(3) fused on VectorE
                    nc.vector.scalar_tensor_tensor(
                        m2v, xv[:, :, :, 3], kst[:, 3:4], m2v, mult, add
                    )

                    # vertical pass: two accumulating matmul passes contract
                    # sel over the row groups; psum[32q+m, (d,j)] = out row
                    # 32d+m of slab s (PE absorbs the final m0+m2 add)
                    for c in range(2):
                        cs = slice(c * 512, (c + 1) * 512)
                        nc.tensor.matmul(
                            pt[32 * q : 32 * q + 32, cs],
                            selt[:],
                            m0[:, cs],
                            start=True,
                            stop=False,
                            tile_position=(0, 32 * q),
                        )
                        nc.tensor.matmul(
                            pt[32 * q : 32 * q + 32, cs],
                            selt[:],
                            m2[:, cs],
                            start=False,
                            stop=True,
                            tile_position=(0, 32 * q),
                        )
                    # keep-warm dummy (result never read)
                    nc.tensor.matmul(
                        warm_pt[:, 0:256],
                        selt[:],
                        warm_src[:],
                        start=True,
                        stop=True,
                    )

                pending.append((g, pt))
                if len(pending) > TAIL_LAG:
                    pg, ppt = pending.pop(0)
                    emit_tail(pg, ppt)

            for pg, ppt in pending:
                emit_tail(pg, ppt)

    # walrus 1-wait-per-instruction workaround, applied at serialization time
    orig = nc.to_json_bytes
    nc.to_json_bytes = lambda: _split_excess_waits(orig())
    return nc


_NC_CACHE: dict[int, bass.Bass] = {}


def _get_nc(rows: int = R) -> bass.Bass:
    if rows not in _NC_CACHE:
        _NC_CACHE[rows] = _build_nc(rows)
    return _NC_CACHE[rows]


def run_spmd(x: np.ndarray, kern: np.ndarray, **spmd_kwargs):
    """Shard, run on 8 cores, gather. Returns (output, BassKernelResults)."""
    assert x.shape == (N, C, H, W) and kern.shape == (F, F)
    x = np.ascontiguousarray(x, dtype=np.float32)
    ks = _make_ks(kern)
    sel = _make_sel()
    nb = N // N_CORES
    in_maps = [
        {"x": x[i * nb : (i + 1) * nb].reshape(R, W), "ks": ks, "sel": sel}
        for i in range(N_CORES)
    ]
    nc = _get_nc()
    res = run_bass_kernel_spmd(
        nc, in_maps, core_ids=list(range(N_CORES)), **spmd_kwargs
    )
    out = np.concatenate(
        [res.results[i]["y"].reshape(nb, C, H // F, WO) for i in range(N_CORES)],
        axis=0,
    )
    return out, res


def kernel(x: np.ndarray, kernel: np.ndarray) -> np.ndarray:
    out, _ = run_spmd(x, kernel)
    return out



# revision 5
# speedup vs baseline: 1.3505x; 1.3505x over previous
"""Trainium2 Bass kernel for nn_Downsampler: depthwise 4x4 conv, stride 4,
VALID padding, one shared (runtime) 4x4 kernel across all channels.

  x: (16, 8, 1024, 1024) f32, kernel: (4, 4) f32 -> out: (16, 8, 256, 256) f32

Sharding: pure data parallel over batch N=16 -> 2 batches per core on 8 cores.

Math: out[o, j] = sum_{di,dj} k[di,dj] * x[4o+di, 4j+dj], rows flattened over
(n, c, h) since every image row has W=1024 and slabs never straddle an (n, c)
boundary (1024 rows per image, slab = 512 rows).

The whole conv runs on the TensorEngine: per slab of 512 input rows (SBUF
tile [128, 4096], partition p, quarter d -> row 512*s + 128*d + p), the
output rows 32*d + m (m = p//4) are

    psum[m, 256*d + j] = sum_dj sum_p selg_dj[p, m] * xt[p, (d, 4j+dj)]

with selg_dj[p, m] = kernel[p%4, dj] * (p//4 == m), i.e. 4 row-quarters x
4 dj-phases = 16 accumulating matmuls, each N=256 with a stride-4 rhs view.
This is exact for an ARBITRARY 4x4 kernel (no separability assumption) and
costs the same PE time as a rank-1 pass because each phase streams 256
columns instead of 1024. float32r runs the PE at 1 col/cycle (plain fp32
matmul lowers to 2 half-speed passes = 4x slower); precision loss is well
inside the 2e-2 gate.

fp32r occupies 2 PE array columns per weight column, so a 32-output matmul
is only placeable at col-group offsets 0 or 64 (s3d3_mm_valid_dst_partition)
-- quarters therefore land side by side in PSUM *columns* of one [32, 1024]
tile (2 banks) at partition base 0, not stacked by tile_position.

Vector/GpSimd do nothing; ScalarE only evicts PSUM -> SBUF (DMA cannot read
PSUM) and issues the output DMA on the ACT HWDGE ring, keeping the SP ring
a pure input stream. Per slab the output is y[128s : 128s+128, :] with row
32d+m taken from ot[m, 256d:256d+256].

Each quarter is its own accumulation group (start on dj=0, stop on dj=3).
A group-start clears has_written bits bank-wide, but the PE executes
matmuls in strict program order, so the earlier quarter sharing the bank is
complete before the clear -- the bits only gate accumulate-vs-overwrite,
and nothing accumulates onto a finished quarter afterwards.

A dummy keep-warm matmul per slab fills the PE's inter-slab wait gaps so
the HAM clock gate stays at K=8/8 (2.4 GHz).
"""

import json
from contextlib import ExitStack

import numpy as np

import concourse.bass as bass
import concourse.mybir as mybir
from concourse.tile import TileContext
from concourse.bass_utils import run_bass_kernel_spmd

N, C, H, W = 16, 8, 1024, 1024
F = 4
N_CORES = 8
R = (N // N_CORES) * C * H  # input rows per core (16384)
WO = W // F  # output row length (256)


def _split_excess_waits(bir_bytes: bytes, max_waits: int = 1) -> bytes:
    """The public neuronxcc walrus supports at most ONE sync wait per
    instruction; hoist excess waits onto NoOps inserted just before."""
    m = json.loads(bir_bytes)

    def fix(blocks):
        for bb in blocks:
            out = []
            for ins in bb.get("instructions", []):
                si = ins.get("sync_info")
                waits = (si or {}).get("on_wait") or []
                if len(waits) > max_waits:
                    extra = waits[:-max_waits]
                    si["on_wait"] = waits[-max_waits:]
                    for i in range(0, len(extra), max_waits):
                        out.append(
                            {
                                "debug": ins.get("debug", 0),
                                "engine": ins["engine"],
                                "ins": [],
                                "outs": [],
                                "name": f"{ins['name']}-ws{i}",
                                "opcode": "NoOp",
                                "sync_info": {
                                    "on_update": [],
                                    "on_wait": extra[i : i + max_waits],
                                },
                            }
                        )
                out.append(ins)
            bb["instructions"] = out
            fix(bb.get("blocks", []))

    for f in m["functions"]:
        fix(f["blocks"])
    return json.dumps(m).encode()


def _make_selg(kernel: np.ndarray) -> np.ndarray:
    """PE stationary weights [128, 4*32]: selg[p, 32*dj + m] =
    kernel[p%4, dj] * (p//4 == m)."""
    kernel = np.asarray(kernel, dtype=np.float32)
    assert kernel.shape == (F, F)
    selg = np.zeros((128, 128), dtype=np.float32)
    p = np.arange(128)
    for dj in range(F):
        selg[p, 32 * dj + p // F] = kernel[p % F, dj]
    return selg


def _build_nc(rows: int, xt_bufs: int = 4, psum_bufs: int = 3, o_bufs: int = 4) -> bass.Bass:
    assert rows % 1024 == 0
    n_slabs = rows // 512

    f32 = mybir.dt.float32
    f32r = mybir.dt.float32r

    nc = bass.Bass("TRN2", target_bir_lowering=False, debug=False)
    x = nc.dram_tensor("x", [rows, W], f32r, kind="ExternalInput")
    selg = nc.dram_tensor("selg", [128, 4 * 32], f32r, kind="ExternalInput")
    y = nc.dram_tensor("y", [rows // F, WO], f32, kind="ExternalOutput")

    with TileContext(nc) as tc:
        with ExitStack() as ctx:
            const_pool = ctx.enter_context(tc.tile_pool(name="const_pool", bufs=1))
            selgt = const_pool.tile([128, 4 * 32], f32r)
            nc.sync.dma_start(selgt[:], selg.ap())

            # keep-warm scratch bank (results never read)
            wp_pool = ctx.enter_context(
                tc.tile_pool(name="wp_pool", bufs=1, space="PSUM")
            )
            warm_pt = wp_pool.tile([32, WO], f32)

            x_pool = ctx.enter_context(tc.tile_pool(name="x_pool", bufs=xt_bufs))
            ps_pool = ctx.enter_context(
                tc.tile_pool(name="ps_pool", bufs=psum_bufs, space="PSUM")
            )
            o_pool = ctx.enter_context(tc.tile_pool(name="o_pool", bufs=o_bufs))

            for s in range(n_slabs):
                if s % 2 == 0:
                    # one input DMA covers TWO slabs (1024 rows, 4 MiB)
                    xt2 = x_pool.tile([128, 8 * W], f32r, name="xt")
                    src = x.ap()[s * 512 : (s + 2) * 512, :].rearrange(
                        "(d p) w -> p d w", p=128
                    )
                    nc.sync.dma_start(
                        xt2[:].rearrange("p (d w) -> p d w", d=8), src
                    )
                half = s % 2
                # xv[p, dd, j, q] = xt2[p, dd*1024 + 4j + q]
                xv = xt2[:].rearrange("p (dd j q) -> p dd j q", dd=8, q=F)

                pt = ps_pool.tile([32, 4 * WO], f32, name="pt")
                for d in range(4):
                    for dj in range(4):
                        nc.tensor.matmul(
                            pt[:, WO * d : WO * d + WO],
                            selgt[:, 32 * dj : 32 * dj + 32],
                            xv[:, 4 * half + d, :, dj],
                            start=(dj == 0),
                            stop=(dj == 3),
                        )
                # keep-warm dummy (result never read; rhs reuses live data)
                nc.tensor.matmul(
                    warm_pt[:, :],
                    selgt[:, 0:32],
                    xv[:, 4 * half, :, 0],
                    start=True,
                    stop=True,
                )

                # evict PSUM -> SBUF (DMA cannot read PSUM), then one output
                # DMA on the ACT HWDGE ring: y row 128s+32d+m <- ot[m, 256d+j]
                ot = o_pool.tile([32, 4 * WO], f32, name="ot")
                nc.scalar.copy(ot[:], pt[:])
                nc.scalar.dma_start(
                    y.ap()[128 * s : 128 * s + 128, :].rearrange(
                        "(d m) j -> m d j", d=4
                    ),
                    ot[:].rearrange("m (d j) -> m d j", d=4),
                )

    # walrus 1-wait-per-instruction workaround, applied at serialization time
    orig = nc.to_json_bytes
    nc.to_json_bytes = lambda: _split_excess_waits(orig())
    return nc


_NC_CACHE: dict[int, bass.Bass] = {}


def _get_nc(rows: int = R) -> bass.Bass:
    if rows not in _NC_CACHE:
        _NC_CACHE[rows] = _build_nc(rows)
    return _NC_CACHE[rows]


def run_spmd(x: np.ndarray, kern: np.ndarray, **spmd_kwargs):
    """Shard, run on 8 cores, gather. Returns (output, BassKernelResults)."""
    assert x.shape == (N, C, H, W) and kern.shape == (F, F)
    x = np.ascontiguousarray(x, dtype=np.float32)
    selg = _make_selg(kern)
    nb = N // N_CORES
    in_maps = [
        {"x": x[i * nb : (i + 1) * nb].reshape(R, W), "selg": selg}
        for i in range(N_CORES)
    ]
    nc = _get_nc()
    res = run_bass_kernel_spmd(
        nc, in_maps, core_ids=list(range(N_CORES)), **spmd_kwargs
    )
    out = np.concatenate(
        [res.results[i]["y"].reshape(nb, C, H // F, WO) for i in range(N_CORES)],
        axis=0,
    )
    return out, res


def kernel(x: np.ndarray, kernel: np.ndarray) -> np.ndarray:
    out, _ = run_spmd(x, kernel)
    return out


# revision 8
# speedup vs baseline: 1.3845x; 1.0252x over previous
"""Trainium2 Bass kernel for nn_Downsampler: depthwise 4x4 conv, stride 4,
VALID padding, one shared (runtime) 4x4 kernel across all channels.

  x: (16, 8, 1024, 1024) f32, kernel: (4, 4) f32 -> out: (16, 8, 256, 256) f32

Sharding: pure data parallel over batch N=16 -> 2 batches per core on 8 cores.

Math: out[o, j] = sum_{di,dj} k[di,dj] * x[4o+di, 4j+dj], rows flattened over
(n, c, h) since every image row has W=1024 and slabs never straddle an (n, c)
boundary (1024 rows per image, slab = 512 rows).

The whole conv runs on the TensorEngine: per slab of 512 input rows (SBUF
tile [128, 4096], partition p, quarter d -> row 512*s + 128*d + p), the
output rows 32*d + m (m = p//4) are

    psum[m, 256*d + j] = sum_dj sum_p selg_dj[p, m] * xt[p, (d, 4j+dj)]

with selg_dj[p, m] = kernel[p%4, dj] * (p//4 == m), i.e. 4 row-quarters x
4 dj-phases = 16 accumulating matmuls, each N=256 with a stride-4 rhs view.
This is exact for an ARBITRARY 4x4 kernel (no separability assumption) and
costs the same PE time as a rank-1 pass because each phase streams 256
columns instead of 1024. float32r runs the PE at 1 col/cycle (plain fp32
matmul lowers to 2 half-speed passes = 4x slower); precision loss is well
inside the 2e-2 gate.

fp32r occupies 2 PE array columns per weight column, so a 32-output matmul
is only placeable at col-group offsets 0 or 64 (s3d3_mm_valid_dst_partition)
-- quarters therefore land side by side in PSUM *columns* of one [32, 1024]
tile (2 banks) at partition base 0, not stacked by tile_position.

Vector/GpSimd do nothing; ScalarE only evicts PSUM -> SBUF (DMA cannot read
PSUM) and issues the output DMA on the ACT HWDGE ring, keeping the SP ring
a pure input stream. Per slab the output is y[128s : 128s+128, :] with row
32d+m taken from ot[m, 256d:256d+256].

Each quarter is its own accumulation group (start on dj=0, stop on dj=3).
A group-start clears has_written bits bank-wide, but the PE executes
matmuls in strict program order, so the earlier quarter sharing the bank is
complete before the clear -- the bits only gate accumulate-vs-overwrite,
and nothing accumulates onto a finished quarter afterwards.

A dummy keep-warm matmul per slab fills the PE's inter-slab wait gaps so
the HAM clock gate stays at K=8/8 (2.4 GHz).
"""

import json
from contextlib import ExitStack

import numpy as np

import concourse.bass as bass
import concourse.mybir as mybir
from concourse.tile import TileContext
from concourse.bass_utils import run_bass_kernel_spmd

N, C, H, W = 16, 8, 1024, 1024
F = 4
N_CORES = 8
R = (N // N_CORES) * C * H  # input rows per core (16384)
WO = W // F  # output row length (256)


def _split_excess_waits(bir_bytes: bytes, max_waits: int = 1) -> bytes:
    """The public neuronxcc walrus supports at most ONE sync wait per
    instruction; hoist excess waits onto NoOps inserted just before."""
    m = json.loads(bir_bytes)

    def fix(blocks):
        for bb in blocks:
            out = []
            for ins in bb.get("instructions", []):
                si = ins.get("sync_info")
                waits = (si or {}).get("on_wait") or []
                if len(waits) > max_waits:
                    extra = waits[:-max_waits]
                    si["on_wait"] = waits[-max_waits:]
                    for i in range(0, len(extra), max_waits):
                        out.append(
                            {
                                "debug": ins.get("debug", 0),
                                "engine": ins["engine"],
                                "ins": [],
                                "outs": [],
                                "name": f"{ins['name']}-ws{i}",
                                "opcode": "NoOp",
                                "sync_info": {
                                    "on_update": [],
                                    "on_wait": extra[i : i + max_waits],
                                },
                            }
                        )
                out.append(ins)
            bb["instructions"] = out
            fix(bb.get("blocks", []))

    for f in m["functions"]:
        fix(f["blocks"])
    return json.dumps(m).encode()


def _make_selg(kernel: np.ndarray) -> np.ndarray:
    """PE stationary weights [128, 4*32]: selg[p, 32*dj + m] =
    kernel[p%4, dj] * (p//4 == m)."""
    kernel = np.asarray(kernel, dtype=np.float32)
    assert kernel.shape == (F, F)
    selg = np.zeros((128, 128), dtype=np.float32)
    p = np.arange(128)
    for dj in range(F):
        selg[p, 32 * dj + p // F] = kernel[p % F, dj]
    return selg


def _build_nc(rows: int, xt_bufs: int = 8, psum_bufs: int = 3, o_bufs: int = 4) -> bass.Bass:
    assert rows % 1024 == 0
    n_slabs = rows // 512

    f32 = mybir.dt.float32
    f32r = mybir.dt.float32r

    nc = bass.Bass("TRN2", target_bir_lowering=False, debug=False)
    x = nc.dram_tensor("x", [rows, W], f32r, kind="ExternalInput")
    selg = nc.dram_tensor("selg", [128, 4 * 32], f32r, kind="ExternalInput")
    y = nc.dram_tensor("y", [rows // F, WO], f32, kind="ExternalOutput")

    with TileContext(nc) as tc:
        with ExitStack() as ctx:
            const_pool = ctx.enter_context(tc.tile_pool(name="const_pool", bufs=1))
            selgt = const_pool.tile([128, 4 * 32], f32r)
            # const load rides the ACT ring so the SP ring is input-only
            nc.scalar.dma_start(selgt[:], selg.ap())

            # keep-warm scratch bank (results never read)
            wp_pool = ctx.enter_context(
                tc.tile_pool(name="wp_pool", bufs=1, space="PSUM")
            )
            warm_pt = wp_pool.tile([32, WO], f32)

            x_pool = ctx.enter_context(tc.tile_pool(name="x_pool", bufs=xt_bufs))
            ps_pool = ctx.enter_context(
                tc.tile_pool(name="ps_pool", bufs=psum_bufs, space="PSUM")
            )
            o_pool = ctx.enter_context(tc.tile_pool(name="o_pool", bufs=o_bufs))

            for s in range(n_slabs):
                # per-slab input DMA (512 rows, 2 MiB): the tail drains at
                # slab granularity instead of stalling on 2-slab tiles
                xt = x_pool.tile([128, 4 * W], f32r, name="xt")
                src = x.ap()[s * 512 : (s + 1) * 512, :].rearrange(
                    "(d p) w -> p d w", p=128
                )
                nc.sync.dma_start(xt[:].rearrange("p (d w) -> p d w", d=4), src)
                # xv[p, d, j, q] = xt[p, d*1024 + 4j + q]
                xv = xt[:].rearrange("p (d j q) -> p d j q", d=4, q=F)

                pt = ps_pool.tile([32, 4 * WO], f32, name="pt")
                for d in range(4):
                    for dj in range(4):
                        nc.tensor.matmul(
                            pt[:, WO * d : WO * d + WO],
                            selgt[:, 32 * dj : 32 * dj + 32],
                            xv[:, d, :, dj],
                            start=(dj == 0),
                            stop=(dj == 3),
                        )
                # keep-warm dummy (result never read; rhs reuses live data)
                nc.tensor.matmul(
                    warm_pt[:, :],
                    selgt[:, 0:32],
                    xv[:, 0, :, 0],
                    start=True,
                    stop=True,
                )

                # evict PSUM -> SBUF (DMA cannot read PSUM), then one output
                # DMA on the ACT HWDGE ring: y row 128s+32d+m <- ot[m, 256d+j]
                ot = o_pool.tile([32, 4 * WO], f32, name="ot")
                nc.scalar.copy(ot[:], pt[:])
                nc.scalar.dma_start(
                    y.ap()[128 * s : 128 * s + 128, :].rearrange(
                        "(d m) j -> m d j", d=4
                    ),
                    ot[:].rearrange("m (d j) -> m d j", d=4),
                )

    # walrus 1-wait-per-instruction workaround, applied at serialization time
    orig = nc.to_json_bytes
    nc.to_json_bytes = lambda: _split_excess_waits(orig())
    return nc


_NC_CACHE: dict[int, bass.Bass] = {}


def _get_nc(rows: int = R) -> bass.Bass:
    if rows not in _NC_CACHE:
        _NC_CACHE[rows] = _build_nc(rows)
    return _NC_CACHE[rows]


def run_spmd(x: np.ndarray, kern: np.ndarray, **spmd_kwargs):
    """Shard, run on 8 cores, gather. Returns (output, BassKernelResults)."""
    assert x.shape == (N, C, H, W) and kern.shape == (F, F)
    x = np.ascontiguousarray(x, dtype=np.float32)
    selg = _make_selg(kern)
    nb = N // N_CORES
    in_maps = [
        {"x": x[i * nb : (i + 1) * nb].reshape(R, W), "selg": selg}
        for i in range(N_CORES)
    ]
    nc = _get_nc()
    res = run_bass_kernel_spmd(
        nc, in_maps, core_ids=list(range(N_CORES)), **spmd_kwargs
    )
    out = np.concatenate(
        [res.results[i]["y"].reshape(nb, C, H // F, WO) for i in range(N_CORES)],
        axis=0,
    )
    return out, res


def kernel(x: np.ndarray, kernel: np.ndarray) -> np.ndarray:
    out, _ = run_spmd(x, kernel)
    return out
